# revision 18
# baseline (speedup 1.0000x reference)
"""Trainium2 Bass kernel for nn_CustomLSTM (B=256, T=1024, I=64, H=256, O=8).

Strategy: data-parallel over batch across 8 cores (32 batch rows each).
Per core the recurrence runs in feature-on-partition layout with two
time-staggered chains of 16 batch rows PACKED into one 32-column block:
pass p holds chain0@t=p in columns 0:16 and chain1@t=p-1 in columns
16:32 (the x stream is host-packed into matching column pairs), so each
weight tile is loaded once per pass and serves both chains with a single
matmul, and the nonlinear phase runs once over the packed block.

Gate columns are packed [c0 c1 f0 f1 | i0 i1 o0 o1] and each pass's
pre-activations land in TWO psum banks (c,f | i,o), so the activation on
the c/f half starts while the PE is still accumulating the i/o half.
sigmoid(g) = (tanh(g/2)+1)/2 with the 1/2 folded into the weights; h is
stored as 2h (Wh, W_ho pre-halved) so one tanh covers all 8 gate tiles.

Wh is stored in fp8e4m3 (x-weights stay bf16): weight loads run at 4
elem/cycle via fast-weight-load, halving the PE-serial weight-load time.
Both weight sets are pre-scaled by 2^WS (undone by the activation input
scale) to keep fp8 values out of the denormal range.

The x-projection (+bias via an appended ones-row) for pass p+2 is
computed between the recurrent blocks of passes p and p+1, giving the PE
work that overlaps the activation/vector tail of each pass.

This file is self-contained: shapes/sharding are hardcoded.
"""

import os
import sys

sys.path.insert(0, "/opt/trn_rl_repo")

import numpy as np

import concourse.bass as bass
import concourse.mybir as mybir
from concourse.tile import TileContext
from concourse.vector_clock import ScopedClock, VectorClock

# ----------------------------------------------------------------------------
# Problem constants (full problem, then per-core)
# ----------------------------------------------------------------------------
B_FULL, T, I, H, O = 256, 1024, 64, 256, 8
NCORES = 8
B = B_FULL // NCORES          # 32 batch rows per core
Bc = B // 2                   # 16 rows per chain
G = 4 * H                     # 1024 gate pre-activations
KT = H // 128                 # 2 k-tiles for the h-part
MT = G // 128                 # 8 m-tiles of gate columns

# m-tile permutation of gate columns: packed slots [c0 c1 f0 f1 i0 i1 o0 o1]
# reference gate column order is [f(0:256) i(256:512) c(512:768) o(768:1024)]
M_PERM = [4, 5, 0, 1, 2, 3, 6, 7]   # source m-tile for each packed slot

WS = 4                        # weight pre-scale 2^WS (fp8 denormal headroom)
L = int(os.environ.get("V2_L", "1"))   # x-projection lookahead in passes
R = L + 2                     # psum ring slots (2 banks each); 2*R <= 8
SC = 64                       # x DMA superchunk (pairs per DMA)
HDT = mybir.dt.bfloat16       # h-state dtype
WH_FP8 = os.environ.get("V2_WH_FP8", "1") == "1"
WHDT = mybir.dt.float8e4 if WH_FP8 else mybir.dt.bfloat16


# ----------------------------------------------------------------------------
# Tile walrus workaround: this container's walrus accepts at most ONE sync
# wait per instruction.  (a) patch the TileContext tail drain to spread its
# waits over per-proc SP nops; (b) after build, hoist excess waits from any
# instruction onto same-engine nops placed immediately before it.
# ----------------------------------------------------------------------------
def _patched_drain_and_barrier(self, tick_clock, wait_clock):
    nc = self.nc
    g = tick_clock.global_clock
    n = len(g)
    for p in range(n):
        if g[p] == 0:
            continue
        vc = VectorClock([g[q] if q == p else 0 for q in range(n)])
        nop = nc.sync.nop(nofuse=True)
        wait_clock.add_sem_waits(nop.ins, ScopedClock({None: vc}))
    nc.sync.drain()
    nc.all_engine_barrier()
    assert self.sems is not None
    popped = nc._tile_sem_poison_stack.pop()
    assert popped is self._sem_poison
    nc.clear_and_free_semaphores(list(self.sems.allocated().values()))
    nc.all_engine_barrier()


def apply_tile_patch():
    TileContext._drain_and_barrier = _patched_drain_and_barrier


def legalize_waits(nc, limit=1):
    """Hoist excess sem waits (>limit per instruction) onto same-engine nops
    inserted immediately before the instruction."""
    eng_builders = {
        mybir.EngineType.PE: nc.tensor,
        mybir.EngineType.DVE: nc.vector,
        mybir.EngineType.Activation: nc.scalar,
        mybir.EngineType.Pool: nc.gpsimd,
        mybir.EngineType.SP: nc.sync,
    }
    n_hoisted = 0
    for f in nc.m.functions:
        for bb in f.blocks:
            snapshot = list(bb.instructions)
            fixes = []  # (index, inst, waits)
            for idx, inst in enumerate(snapshot):
                si = inst.sync_info
                waits = list(si.on_wait) if si and si.on_wait else []
                if len(waits) > limit:
                    fixes.append((idx, inst, waits))
            if not fixes:
                continue
            out = []
            prev = 0
            for idx, inst, waits in fixes:
                out.extend(snapshot[prev:idx])
                keep = waits[-limit:]
                excess = waits[:-limit]
                for w in excess:
                    builder = eng_builders[inst.engine]
                    nop_bi = builder.nop(nofuse=True)
                    nop_inst = nop_bi.ins
                    cur = nc.cur_bb.bb
                    assert cur.instructions[-1] is nop_inst
                    cur.instructions.pop()
                    nop_inst.sync_info = mybir.SyncInfo(on_wait=[w], on_update=[])
                    out.append(nop_inst)
                    n_hoisted += 1
                inst.sync_info = mybir.SyncInfo(
                    on_wait=keep, on_update=list(inst.sync_info.on_update or [])
                )
                out.append(inst)
                prev = idx + 1
            out.extend(snapshot[prev:])
            bb.instructions = out
    return n_hoisted


# ----------------------------------------------------------------------------
# Kernel build
# ----------------------------------------------------------------------------
def build_nc(t_steps=T):
    """Build the per-core Bass program. Returns nc."""
    apply_tile_patch()
    fp32 = mybir.dt.float32
    bf16 = mybir.dt.bfloat16
    Af = mybir.ActivationFunctionType
    mlt = mybir.AluOpType.mult
    addo = mybir.AluOpType.add
    iscale = float(2.0 ** (-WS))

    n_pass = t_steps + 1          # pass p: chain0@t=p (cols 0:16),
                                  #         chain1@t=p-1 (cols 16:32)
    nc = bass.Bass()
    xT_d = nc.dram_tensor("xT", [I + 1, n_pass * B], bf16, kind="ExternalInput")
    Wh_d = nc.dram_tensor("Wh", [128, KT * G], WHDT, kind="ExternalInput")
    Wx_d = nc.dram_tensor("Wx", [I + 1, G], bf16, kind="ExternalInput")
    Who_d = nc.dram_tensor("Who", [128, KT * O], fp32, kind="ExternalInput")
    y_d = nc.dram_tensor("y", [B, O], fp32, kind="ExternalOutput")

    with TileContext(nc) as tc:
        with (
            tc.tile_pool(name="wpool", bufs=1) as wpool,
            tc.tile_pool(name="state", bufs=1) as state,
            tc.tile_pool(name="xbuf", bufs=3) as xbuf,
            tc.tile_pool(name="gbuf", bufs=3) as gbuf,
            tc.tile_pool(name="tbuf", bufs=3) as tbuf,
            tc.tile_pool(name="ring", bufs=1, space="PSUM") as ringp,
            tc.tile_pool(name="ysb", bufs=1) as ysbp,
        ):
            # --- weights ---
            Wh_s = wpool.tile([128, KT * G], WHDT, tag="Wh_s")
            nc.sync.dma_start(Wh_s[:], Wh_d[:])
            Wx_s = wpool.tile([I + 1, G], bf16, tag="Wx_s")
            nc.sync.dma_start(Wx_s[:], Wx_d[:])
            Who_s = wpool.tile([128, KT * O], fp32, tag="Who_s")
            nc.sync.dma_start(Who_s[:], Who_d[:])

            # --- packed state: cols 0:16 chain0, 16:32 chain1 ---
            h_all = state.tile([128, KT, B], HDT, tag="h_all")   # = 2h
            c_all = state.tile([128, KT, B], fp32, tag="c_all")
            nc.vector.memset(h_all[:], 0.0)
            nc.vector.memset(c_all[:], 0.0)

            # --- psum ring: R slots x 2 banks (gate groups c,f | i,o) ---
            rA = [ringp.tile([128, 4, B], fp32, tag=f"rA{r}", name=f"rA{r}")
                  for r in range(R)]
            rB = [ringp.tile([128, 4, B], fp32, tag=f"rB{r}", name=f"rB{r}")
                  for r in range(R)]
            # v2 / th live in PSUM: the scalar engine's PSUM access is
            # faster than SBUF on both its read (th's input) and write
            # (th itself), and both sit on the serial per-pass cycle.
            v2p = ringp.tile([128, KT, B], fp32, tag="v2p", name="v2p")
            thp = ringp.tile([128, KT, B], fp32, tag="thp", name="thp")

            # --- x superchunks, DMA'd ahead ---
            n_chunk = (n_pass + SC - 1) // SC
            xch = {}

            def fetch_chunk(ci):
                if ci in xch or ci >= n_chunk:
                    return
                cols = min(SC, n_pass - ci * SC) * B
                xt = xbuf.tile([I + 1, SC * B], bf16, tag="xch")
                nc.sync.dma_start(
                    xt[:, 0:cols], xT_d[:, ci * SC * B : ci * SC * B + cols]
                )
                xch[ci] = xt

            fetch_chunk(0)
            fetch_chunk(1)

            def x_block(q):
                """x-projection (+bias) of pair q into ring slot q%R."""
                if q % SC == 0:
                    fetch_chunk(q // SC + 1)
                xt = xch[q // SC]
                rhs = xt[:, (q % SC) * B : (q % SC) * B + B]
                for m in range(MT):
                    dst = rA[q % R][:, m, :] if m < 4 else rB[q % R][:, m - 4, :]
                    nc.tensor.matmul(
                        dst,
                        Wx_s[:, m * 128 : (m + 1) * 128],
                        rhs,
                        start=(m % 4 == 0),   # first write into each bank
                        stop=False,
                        skip_group_check=True,
                    )

            # prologue: fill the x pipeline
            for q in range(min(L, n_pass)):
                x_block(q)

            # --- main loop ---
            for p in range(n_pass):
                if p + L < n_pass:
                    x_block(p + L)

                # recurrent accumulation: m-outer, k-inner; c,f bank first
                sA, sB = rA[p % R], rB[p % R]
                for m in range(MT):
                    dst = sA[:, m, :] if m < 4 else sB[:, m - 4, :]
                    for k in range(KT):
                        nc.tensor.matmul(
                            dst,
                            Wh_s[:, k * G + m * 128 : k * G + (m + 1) * 128],
                            h_all[:, k, :],
                            start=False,
                            stop=(k == KT - 1 and m in (3, 7)),
                            skip_group_check=True,
                        )

                # nonlinear phase on the packed block. Last pass updates only
                # chain1 (cols 16:32) so chain0's final state survives.
                lo = Bc if p == t_steps else 0
                cs = slice(lo, B)
                g8 = gbuf.tile([128, MT, B], fp32, tag="g8")
                u1 = tbuf.tile([128, KT, B], fp32, tag="u1")
                u2 = tbuf.tile([128, KT, B], fp32, tag="u2")
                v2 = v2p
                th = thp
                # tanh over c,f banks as soon as they are accumulated; the
                # i,o bank follows.  tc/tf/ti/to are tanh(g/2); chat is true
                # tanh (c columns unhalved).
                nc.scalar.activation(g8[:, 0:4, cs], sA[:, :, cs], Af.Tanh,
                                     scale=iscale)
                # u1 = (tf+1)*c = 2*f*c  (early; off the critical tail)
                nc.vector.scalar_tensor_tensor(
                    u1[:, :, cs], g8[:, 2:4, cs], 1.0, c_all[:, :, cs],
                    addo, mlt)
                nc.scalar.activation(g8[:, 4:8, cs], sB[:, :, cs], Af.Tanh,
                                     scale=iscale)
                # u2 = (ti+1)*chat = 2*i*chat
                nc.vector.scalar_tensor_tensor(
                    u2[:, :, cs], g8[:, 4:6, cs], 1.0, g8[:, 0:2, cs],
                    addo, mlt)
                nc.vector.tensor_add(v2[:, :, cs], u1[:, :, cs], u2[:, :, cs])
                # th = tanh(c_new) via free input scale (v2 = 2*c_new)
                nc.scalar.activation(th[:, :, cs], v2[:, :, cs], Af.Tanh,
                                     scale=0.5)
                # h2 = (to+1)*th = 2*o*tanh(c_new) = 2h
                nc.vector.scalar_tensor_tensor(
                    h_all[:, :, cs], g8[:, 6:8, cs], 1.0, th[:, :, cs],
                    addo, mlt)
                # true c for the next pass (off the critical path)
                nc.vector.tensor_scalar_mul(c_all[:, :, cs], v2[:, :, cs], 0.5)

            # --- output projection: y = h_T @ W_ho (bias on host) ---
            hc = ysbp.tile([128, KT, B], fp32, tag="hc", name="hc")
            nc.vector.tensor_copy(hc[:], h_all[:])
            yp = rA[0][0:B, 0, 0:O]
            for k in range(KT):
                nc.tensor.matmul(
                    yp[:],
                    hc[:, k, :],
                    Who_s[:, k * O : (k + 1) * O],
                    start=(k == 0),
                    stop=(k == KT - 1),
                    skip_group_check=True,
                )
            ys = ysbp.tile([B, O], fp32, tag="ys")
            nc.vector.tensor_copy(ys[:], yp[:])
            nc.sync.dma_start(y_d[:], ys[:])

    legalize_waits(nc, limit=1)
    return nc


# ----------------------------------------------------------------------------
# Host-side packing
# ----------------------------------------------------------------------------
def pack_weights(W_f, b_f, W_i, b_i, W_c, b_c, W_o, b_o, W_ho, t_steps=T):
    """Build Wh [128, KT*G] (fp8/bf16), Wx [I+1, G] bf16, Who [128, KT*O]."""
    import ml_dtypes

    np_wh = ml_dtypes.float8_e4m3 if WH_FP8 else ml_dtypes.bfloat16
    Wg = np.concatenate([W_f, W_i, W_c, W_o], axis=1).astype(np.float32)
    bg = np.concatenate([b_f, b_i, b_c, b_o], axis=0).astype(np.float32)
    cols = np.concatenate([np.arange(m * 128, (m + 1) * 128) for m in M_PERM])
    Wg_p = Wg[:, cols]
    bg_p = bg[cols]
    # tanh trick: halve the sigmoid-gate columns (packed slots 2:8 = f,i,o);
    # h stored as 2h so all Wh rows and W_ho halve again.  2^WS pre-scale on
    # both gate weight sets (undone by the activation input scale).
    colscale = np.full((G,), 0.5, np.float32) * (2.0 ** WS)
    colscale[0 : 2 * 128] = 1.0 * (2.0 ** WS)
    Wh = Wg_p[0:H, :] * colscale[None, :] * 0.5
    Wx = Wg_p[H : H + I, :] * colscale[None, :]
    bgs = bg_p * colscale
    Who = W_ho.astype(np.float32) * 0.5
    Wx_aug = np.concatenate([Wx, bgs[None, :]], axis=0)        # [65, G]
    Wh_pk = np.concatenate(
        [Wh[k * 128 : (k + 1) * 128, :] for k in range(KT)], axis=1
    )
    Who_pk = np.concatenate(
        [Who[k * 128 : (k + 1) * 128, :] for k in range(KT)], axis=1
    )
    return (Wh_pk.astype(np_wh), Wx_aug.astype(ml_dtypes.bfloat16),
            Who_pk.astype(np.float32))


def pack_x(x, t_steps=T):
    """x [B_FULL, T, I] -> per-core xT [I+1, (T+1)*B] bf16 pair-packed:
    pair p cols 0:16 = x_t=p rows 0:16 (zero at p=T), cols 16:32 = x_t=p-1
    rows 16:32 (zero at p=0); ones-row bias zeroed on invalid halves."""
    import ml_dtypes

    n_pass = t_steps + 1
    outs = []
    for c in range(NCORES):
        xs = np.asarray(x[c * B : (c + 1) * B, :t_steps, :], dtype=np.float32)
        xt = np.ascontiguousarray(xs.transpose(2, 1, 0))       # [I, T, B]
        xp = np.zeros((I + 1, n_pass, B), np.float32)
        xp[0:I, 0:t_steps, 0:Bc] = xt[:, :, 0:Bc]
        xp[0:I, 1:n_pass, Bc:B] = xt[:, :, Bc:B]
        xp[I, 0:t_steps, 0:Bc] = 1.0
        xp[I, 1:n_pass, Bc:B] = 1.0
        outs.append(xp.reshape(I + 1, n_pass * B).astype(ml_dtypes.bfloat16))
    return outs


# ----------------------------------------------------------------------------
# Public entry point
# ----------------------------------------------------------------------------
_CACHE = {}


def _get_nc(t_steps=T):
    key = (t_steps, str(WHDT))
    if key not in _CACHE:
        _CACHE[key] = build_nc(t_steps)
    return _CACHE[key]


def kernel(x, W_f, b_f, W_i, b_i, W_c, b_c, W_o, b_o, W_ho, b_ho):
    from concourse.bass_utils import run_bass_kernel_spmd

    x = np.asarray(x)
    nc = _get_nc()
    Wh_pk, Wx_aug, Who_pk = pack_weights(
        W_f, b_f, W_i, b_i, W_c, b_c, W_o, b_o, W_ho
    )
    xs = pack_x(x)
    in_maps = [
        {"xT": xs[c], "Wh": Wh_pk, "Wx": Wx_aug, "Who": Who_pk}
        for c in range(NCORES)
    ]
    res = run_bass_kernel_spmd(nc, in_maps, list(range(NCORES)))
    y = np.concatenate([res.results[c]["y"] for c in range(NCORES)], axis=0)
    return (y + np.asarray(b_ho, np.float32)[None, :]).astype(np.float32)


# revision 19
# speedup vs baseline: 1.4572x; 1.4572x over previous
"""Trainium2 Bass kernel for nn_CustomLSTM (B=256, T=1024, I=64, H=256, O=8).

Strategy: data-parallel over batch across 8 cores (32 batch rows each).
Per core the recurrence runs in feature-on-partition layout with two
time-staggered chains of 16 batch rows PACKED into one 32-column block:
pass p holds chain0@t=p in columns 0:16 and chain1@t=p-1 in columns
16:32 (the x stream is host-packed into matching column pairs), so each
weight tile is loaded once per pass and serves both chains with a single
matmul, and the nonlinear phase runs once over the packed block.

Gate columns are packed [c0 c1 f0 f1 | i0 i1 o0 o1] and each pass's
pre-activations land in TWO psum banks (c,f | i,o), so the activation on
the c/f half starts while the PE is still accumulating the i/o half.
sigmoid(g) = (tanh(g/2)+1)/2 with the 1/2 folded into the weights; h is
stored as 2h (Wh, W_ho pre-halved) so one tanh covers all 8 gate tiles.

Wh is stored in fp8e4m3 (x-weights stay bf16): weight loads run at 4
elem/cycle via fast-weight-load, halving the PE-serial weight-load time.
Both weight sets are pre-scaled by 2^WS (undone by the activation input
scale) to keep fp8 values out of the denormal range.

The x-projection (+bias via an appended ones-row) for pass p+2 is
computed between the recurrent blocks of passes p and p+1, giving the PE
work that overlaps the activation/vector tail of each pass.

This file is self-contained: shapes/sharding are hardcoded.
"""

import os
import sys

sys.path.insert(0, "/opt/trn_rl_repo")

import numpy as np

import concourse.bass as bass
import concourse.mybir as mybir
from concourse.tile import TileContext
from concourse.vector_clock import ScopedClock, VectorClock

# ----------------------------------------------------------------------------
# Problem constants (full problem, then per-core)
# ----------------------------------------------------------------------------
B_FULL, T, I, H, O = 256, 1024, 64, 256, 8
NCORES = 8
B = B_FULL // NCORES          # 32 batch rows per core
Bc = B // 2                   # 16 rows per chain
G = 4 * H                     # 1024 gate pre-activations
KT = H // 128                 # 2 k-tiles for the h-part
MT = G // 128                 # 8 m-tiles of gate columns

# m-tile permutation of gate columns: packed slots [c0 c1 f0 f1 i0 i1 o0 o1]
# reference gate column order is [f(0:256) i(256:512) c(512:768) o(768:1024)]
M_PERM = [4, 5, 0, 1, 2, 3, 6, 7]   # source m-tile for each packed slot

WS = 4                        # weight pre-scale 2^WS (fp8 denormal headroom)
L = int(os.environ.get("V2_L", "1"))   # x-projection lookahead in passes
R = L + 2                     # psum ring slots (2 banks each); 2*R <= 8
SC = 64                       # x DMA superchunk (pairs per DMA)
HDT = mybir.dt.bfloat16       # h-state dtype
WH_FP8 = os.environ.get("V2_WH_FP8", "1") == "1"
WHDT = mybir.dt.float8e4 if WH_FP8 else mybir.dt.bfloat16


# ----------------------------------------------------------------------------
# Tile walrus workaround: this container's walrus accepts at most ONE sync
# wait per instruction.  (a) patch the TileContext tail drain to spread its
# waits over per-proc SP nops; (b) after build, hoist excess waits from any
# instruction onto same-engine nops placed immediately before it.
# ----------------------------------------------------------------------------
def _patched_drain_and_barrier(self, tick_clock, wait_clock):
    nc = self.nc
    g = tick_clock.global_clock
    n = len(g)
    for p in range(n):
        if g[p] == 0:
            continue
        vc = VectorClock([g[q] if q == p else 0 for q in range(n)])
        nop = nc.sync.nop(nofuse=True)
        wait_clock.add_sem_waits(nop.ins, ScopedClock({None: vc}))
    nc.sync.drain()
    nc.all_engine_barrier()
    assert self.sems is not None
    popped = nc._tile_sem_poison_stack.pop()
    assert popped is self._sem_poison
    nc.clear_and_free_semaphores(list(self.sems.allocated().values()))
    nc.all_engine_barrier()


def apply_tile_patch():
    TileContext._drain_and_barrier = _patched_drain_and_barrier


def legalize_waits(nc, limit=1):
    """Hoist excess sem waits (>limit per instruction) onto same-engine nops
    inserted immediately before the instruction."""
    eng_builders = {
        mybir.EngineType.PE: nc.tensor,
        mybir.EngineType.DVE: nc.vector,
        mybir.EngineType.Activation: nc.scalar,
        mybir.EngineType.Pool: nc.gpsimd,
        mybir.EngineType.SP: nc.sync,
    }
    n_hoisted = 0
    for f in nc.m.functions:
        for bb in f.blocks:
            snapshot = list(bb.instructions)
            fixes = []  # (index, inst, waits)
            for idx, inst in enumerate(snapshot):
                si = inst.sync_info
                waits = list(si.on_wait) if si and si.on_wait else []
                if len(waits) > limit:
                    fixes.append((idx, inst, waits))
            if not fixes:
                continue
            out = []
            prev = 0
            for idx, inst, waits in fixes:
                out.extend(snapshot[prev:idx])
                keep = waits[-limit:]
                excess = waits[:-limit]
                for w in excess:
                    builder = eng_builders[inst.engine]
                    nop_bi = builder.nop(nofuse=True)
                    nop_inst = nop_bi.ins
                    cur = nc.cur_bb.bb
                    assert cur.instructions[-1] is nop_inst
                    cur.instructions.pop()
                    nop_inst.sync_info = mybir.SyncInfo(on_wait=[w], on_update=[])
                    out.append(nop_inst)
                    n_hoisted += 1
                inst.sync_info = mybir.SyncInfo(
                    on_wait=keep, on_update=list(inst.sync_info.on_update or [])
                )
                out.append(inst)
                prev = idx + 1
            out.extend(snapshot[prev:])
            bb.instructions = out
    return n_hoisted


# ----------------------------------------------------------------------------
# Kernel build
# ----------------------------------------------------------------------------
def build_nc(t_steps=T):
    """Build the per-core Bass program. Returns nc."""
    apply_tile_patch()
    fp32 = mybir.dt.float32
    bf16 = mybir.dt.bfloat16
    Af = mybir.ActivationFunctionType
    mlt = mybir.AluOpType.mult
    addo = mybir.AluOpType.add
    iscale = float(2.0 ** (-WS))

    n_pass = t_steps + 1          # pass p: chain0@t=p (cols 0:16),
                                  #         chain1@t=p-1 (cols 16:32)
    nc = bass.Bass()
    xT_d = nc.dram_tensor("xT", [I + 1, n_pass * B], bf16, kind="ExternalInput")
    Wh_d = nc.dram_tensor("Wh", [128, KT * G], WHDT, kind="ExternalInput")
    Wx_d = nc.dram_tensor("Wx", [I + 1, G], bf16, kind="ExternalInput")
    Who_d = nc.dram_tensor("Who", [128, KT * O], fp32, kind="ExternalInput")
    y_d = nc.dram_tensor("y", [B, O], fp32, kind="ExternalOutput")

    with TileContext(nc) as tc:
        with (
            tc.tile_pool(name="wpool", bufs=1) as wpool,
            tc.tile_pool(name="state", bufs=1) as state,
            tc.tile_pool(name="xbuf", bufs=3) as xbuf,
            tc.tile_pool(name="gbuf", bufs=3) as gbuf,
            tc.tile_pool(name="tbuf", bufs=3) as tbuf,
            tc.tile_pool(name="ring", bufs=1, space="PSUM") as ringp,
            tc.tile_pool(name="ysb", bufs=1) as ysbp,
        ):
            # --- weights ---
            Wh_s = wpool.tile([128, KT * G], WHDT, tag="Wh_s")
            nc.sync.dma_start(Wh_s[:], Wh_d[:])
            Wx_s = wpool.tile([I + 1, G], bf16, tag="Wx_s")
            nc.sync.dma_start(Wx_s[:], Wx_d[:])
            Who_s = wpool.tile([128, KT * O], fp32, tag="Who_s")
            nc.sync.dma_start(Who_s[:], Who_d[:])

            # --- packed state: cols 0:16 chain0, 16:32 chain1 ---
            h_all = state.tile([128, KT, B], HDT, tag="h_all")   # = 2h
            c_all = state.tile([128, KT, B], fp32, tag="c_all")
            nc.vector.memset(h_all[:], 0.0)
            nc.vector.memset(c_all[:], 0.0)

            # --- psum ring: R slots x 2 banks (gate groups c,f | i,o) ---
            rA = [ringp.tile([128, 4, B], fp32, tag=f"rA{r}", name=f"rA{r}")
                  for r in range(R)]
            rB = [ringp.tile([128, 4, B], fp32, tag=f"rB{r}", name=f"rB{r}")
                  for r in range(R)]
            # v2 / th live in PSUM: the scalar engine's PSUM access is
            # faster than SBUF on both its read (th's input) and write
            # (th itself), and both sit on the serial per-pass cycle.
            v2p = ringp.tile([128, KT, B], fp32, tag="v2p", name="v2p")
            thp = ringp.tile([128, KT, B], fp32, tag="thp", name="thp")

            # --- x superchunks, DMA'd ahead ---
            n_chunk = (n_pass + SC - 1) // SC
            xch = {}

            def fetch_chunk(ci):
                if ci in xch or ci >= n_chunk:
                    return
                cols = min(SC, n_pass - ci * SC) * B
                xt = xbuf.tile([I + 1, SC * B], bf16, tag="xch")
                nc.sync.dma_start(
                    xt[:, 0:cols], xT_d[:, ci * SC * B : ci * SC * B + cols]
                )
                xch[ci] = xt

            fetch_chunk(0)
            fetch_chunk(1)

            def x_block(q):
                """x-projection (+bias) of pair q into ring slot q%R."""
                if q % SC == 0:
                    fetch_chunk(q // SC + 1)
                xt = xch[q // SC]
                rhs = xt[:, (q % SC) * B : (q % SC) * B + B]
                for m in range(MT):
                    dst = rA[q % R][:, m, :] if m < 4 else rB[q % R][:, m - 4, :]
                    nc.tensor.matmul(
                        dst,
                        Wx_s[:, m * 128 : (m + 1) * 128],
                        rhs,
                        start=(m % 4 == 0),   # first write into each bank
                        stop=False,
                        skip_group_check=True,
                    )

            # prologue: fill the x pipeline
            for q in range(min(L, n_pass)):
                x_block(q)

            # --- main loop ---
            for p in range(n_pass):
                if p + L < n_pass:
                    x_block(p + L)

                # recurrent accumulation: m-outer, k-inner; c,f bank first
                sA, sB = rA[p % R], rB[p % R]
                for m in range(MT):
                    dst = sA[:, m, :] if m < 4 else sB[:, m - 4, :]
                    for k in range(KT):
                        nc.tensor.matmul(
                            dst,
                            Wh_s[:, k * G + m * 128 : k * G + (m + 1) * 128],
                            h_all[:, k, :],
                            start=False,
                            stop=(k == KT - 1 and m in (3, 7)),
                            skip_group_check=True,
                        )

                # nonlinear phase on the packed block. Last pass updates only
                # chain1 (cols 16:32) so chain0's final state survives.
                lo = Bc if p == t_steps else 0
                cs = slice(lo, B)
                g8 = gbuf.tile([128, MT, B], fp32, tag="g8")
                u1 = tbuf.tile([128, KT, B], fp32, tag="u1")
                u2 = tbuf.tile([128, KT, B], fp32, tag="u2")
                v2 = v2p
                th = thp
                # tanh over c,f banks as soon as they are accumulated; the
                # i,o bank follows.  tc/tf/ti/to are tanh(g/2); chat is true
                # tanh (c columns unhalved).
                nc.scalar.activation(g8[:, 0:4, cs], sA[:, :, cs], Af.Tanh,
                                     scale=iscale)
                # u1 = (tf+1)*c = 2*f*c  (early; off the critical tail)
                nc.vector.scalar_tensor_tensor(
                    u1[:, :, cs], g8[:, 2:4, cs], 1.0, c_all[:, :, cs],
                    addo, mlt)
                # i-gate tanh first (unblocks u2 with a smaller op);
                # o-gate tanh after -- its consumer h2 runs much later.
                nc.scalar.activation(g8[:, 4:6, cs], sB[:, 0:2, cs], Af.Tanh,
                                     scale=iscale)
                nc.scalar.activation(g8[:, 6:8, cs], sB[:, 2:4, cs], Af.Tanh,
                                     scale=iscale)
                # u2 = (ti+1)*chat = 2*i*chat
                nc.vector.scalar_tensor_tensor(
                    u2[:, :, cs], g8[:, 4:6, cs], 1.0, g8[:, 0:2, cs],
                    addo, mlt)
                nc.vector.tensor_add(v2[:, :, cs], u1[:, :, cs], u2[:, :, cs])
                # th = tanh(c_new) via free input scale (v2 = 2*c_new)
                nc.scalar.activation(th[:, :, cs], v2[:, :, cs], Af.Tanh,
                                     scale=0.5)
                # h2 = (to+1)*th = 2*o*tanh(c_new) = 2h
                nc.vector.scalar_tensor_tensor(
                    h_all[:, :, cs], g8[:, 6:8, cs], 1.0, th[:, :, cs],
                    addo, mlt)
                # true c for the next pass (off the critical path)
                nc.vector.tensor_scalar_mul(c_all[:, :, cs], v2[:, :, cs], 0.5)

            # --- output projection: y = h_T @ W_ho (bias on host) ---
            hc = ysbp.tile([128, KT, B], fp32, tag="hc", name="hc")
            nc.vector.tensor_copy(hc[:], h_all[:])
            yp = rA[0][0:B, 0, 0:O]
            for k in range(KT):
                nc.tensor.matmul(
                    yp[:],
                    hc[:, k, :],
                    Who_s[:, k * O : (k + 1) * O],
                    start=(k == 0),
                    stop=(k == KT - 1),
                    skip_group_check=True,
                )
            ys = ysbp.tile([B, O], fp32, tag="ys")
            nc.vector.tensor_copy(ys[:], yp[:])
            nc.sync.dma_start(y_d[:], ys[:])

    legalize_waits(nc, limit=1)
    return nc


# ----------------------------------------------------------------------------
# Host-side packing
# ----------------------------------------------------------------------------
def pack_weights(W_f, b_f, W_i, b_i, W_c, b_c, W_o, b_o, W_ho, t_steps=T):
    """Build Wh [128, KT*G] (fp8/bf16), Wx [I+1, G] bf16, Who [128, KT*O]."""
    import ml_dtypes

    np_wh = ml_dtypes.float8_e4m3 if WH_FP8 else ml_dtypes.bfloat16
    Wg = np.concatenate([W_f, W_i, W_c, W_o], axis=1).astype(np.float32)
    bg = np.concatenate([b_f, b_i, b_c, b_o], axis=0).astype(np.float32)
    cols = np.concatenate([np.arange(m * 128, (m + 1) * 128) for m in M_PERM])
    Wg_p = Wg[:, cols]
    bg_p = bg[cols]
    # tanh trick: halve the sigmoid-gate columns (packed slots 2:8 = f,i,o);
    # h stored as 2h so all Wh rows and W_ho halve again.  2^WS pre-scale on
    # both gate weight sets (undone by the activation input scale).
    colscale = np.full((G,), 0.5, np.float32) * (2.0 ** WS)
    colscale[0 : 2 * 128] = 1.0 * (2.0 ** WS)
    Wh = Wg_p[0:H, :] * colscale[None, :] * 0.5
    Wx = Wg_p[H : H + I, :] * colscale[None, :]
    bgs = bg_p * colscale
    Who = W_ho.astype(np.float32) * 0.5
    Wx_aug = np.concatenate([Wx, bgs[None, :]], axis=0)        # [65, G]
    Wh_pk = np.concatenate(
        [Wh[k * 128 : (k + 1) * 128, :] for k in range(KT)], axis=1
    )
    Who_pk = np.concatenate(
        [Who[k * 128 : (k + 1) * 128, :] for k in range(KT)], axis=1
    )
    return (Wh_pk.astype(np_wh), Wx_aug.astype(ml_dtypes.bfloat16),
            Who_pk.astype(np.float32))


def pack_x(x, t_steps=T):
    """x [B_FULL, T, I] -> per-core xT [I+1, (T+1)*B] bf16 pair-packed:
    pair p cols 0:16 = x_t=p rows 0:16 (zero at p=T), cols 16:32 = x_t=p-1
    rows 16:32 (zero at p=0); ones-row bias zeroed on invalid halves."""
    import ml_dtypes

    n_pass = t_steps + 1
    outs = []
    for c in range(NCORES):
        xs = np.asarray(x[c * B : (c + 1) * B, :t_steps, :], dtype=np.float32)
        xt = np.ascontiguousarray(xs.transpose(2, 1, 0))       # [I, T, B]
        xp = np.zeros((I + 1, n_pass, B), np.float32)
        xp[0:I, 0:t_steps, 0:Bc] = xt[:, :, 0:Bc]
        xp[0:I, 1:n_pass, Bc:B] = xt[:, :, Bc:B]
        xp[I, 0:t_steps, 0:Bc] = 1.0
        xp[I, 1:n_pass, Bc:B] = 1.0
        outs.append(xp.reshape(I + 1, n_pass * B).astype(ml_dtypes.bfloat16))
    return outs


# ----------------------------------------------------------------------------
# Public entry point
# ----------------------------------------------------------------------------
_CACHE = {}


def _get_nc(t_steps=T):
    key = (t_steps, str(WHDT))
    if key not in _CACHE:
        _CACHE[key] = build_nc(t_steps)
    return _CACHE[key]


def kernel(x, W_f, b_f, W_i, b_i, W_c, b_c, W_o, b_o, W_ho, b_ho):
    from concourse.bass_utils import run_bass_kernel_spmd

    x = np.asarray(x)
    nc = _get_nc()
    Wh_pk, Wx_aug, Who_pk = pack_weights(
        W_f, b_f, W_i, b_i, W_c, b_c, W_o, b_o, W_ho
    )
    xs = pack_x(x)
    in_maps = [
        {"xT": xs[c], "Wh": Wh_pk, "Wx": Wx_aug, "Who": Who_pk}
        for c in range(NCORES)
    ]
    res = run_bass_kernel_spmd(nc, in_maps, list(range(NCORES)))
    y = np.concatenate([res.results[c]["y"] for c in range(NCORES)], axis=0)
    return (y + np.asarray(b_ho, np.float32)[None, :]).astype(np.float32)
